# revision 33
# baseline (speedup 1.0000x reference)
"""Causal self-attention (RMSNorm + QKV + causal attention + out-proj) on 8 trn2
NeuronCores.

Sharding: core c handles batch b=c//2 and head-group g=c%2 (Megatron-style TP
over the 16 heads: 8 heads per group). Each core computes a partial output
y_part[b,g] = attn_out(heads of g) @ w_proj[:, g-cols].T ; the host sums the two
TP partials per batch.

On-core pipeline (T=1024, E=1024, 8 local heads, D=64), all matmuls bf16 with
fp32 PSUM accumulation, laid out to keep the PE continuously busy (p-state):
  1. r[t] = rsqrt(mean_e x^2 + eps): ACT squares + DVE add tree + ones-matmul
     partition reduce + fast-approx reciprocal; r broadcast to 128 partitions
     (K=1 matmul) and transposed to per-t-tile columns rT (PE transpose).
  2. qk^T[f,t] = (Wqk x^T) * r[t] (RMSNorm folded into eviction); V[t,f] =
     (x^T)^T W_v^T scaled by rT at eviction (tensor_scalar), stored with a
     ones column per head (65 cols) so attention also yields softmax denoms.
  3. Per head: causal additive masks are prefilled into PSUM (Pool/DVE), the
     S^T matmul accumulates on top (start=False), EXP narrowed to the valid
     column range; O^T (+denominator row) = V'.T @ P; normalization via
     fast-approx reciprocal + K=1 broadcast matmul. Head pairs share 128-row
     ot tiles. Software-pipelined (S of head i ahead of O of i-1, norm i-2).
  4. y[t,:] = sum of 4 K=128 matmuls over head-pair tiles, evicted via ACT
     copy and DMA'd out in fp32.
"""

import sys

sys.path.insert(0, "/opt/trn_rl_repo")

import numpy as np
import ml_dtypes

import concourse.bass as bass
import concourse.tile as tile
from concourse import mybir
from concourse.bass_utils import run_bass_kernel_spmd

BF16 = ml_dtypes.bfloat16

B, T, E, H = 4, 1024, 1024, 16
D = E // H  # 64
HL = 8  # heads per core (TP degree 2)
FL = HL * D  # 512 local head cols
EPS = 1e-5
N_CORES = 8
NEG = -2400.0  # additive mask; exp(0.125 * -2400) underflows fp32 to 0

AF = mybir.ActivationFunctionType
DT = mybir.dt


# ---------------------------------------------------------------------------
# Walrus in this toolchain rejects instructions whose tail Drain carries more
# than one semaphore wait; split the TileContext exit drain into chained
# single-wait drains.
def _patched_drain_and_barrier(self, tick_clock, wait_clock):
    nc = self.nc
    drain_inst = nc.sync.drain()
    wait_clock.add_sem_waits(
        drain_inst.ins, tile.ScopedClock({None: tick_clock.global_clock})
    )
    mi = drain_inst.ins
    si = mi.sync_info
    if si is not None and len(si.on_wait) > 1:
        waits = list(si.on_wait)
        mi.sync_info = mybir.SyncInfo(on_wait=waits[:1], on_update=list(si.on_update))
        for w in waits[1:]:
            extra = nc.sync.drain().ins
            extra.sync_info = mybir.SyncInfo(on_wait=[w], on_update=[])
    nc.all_engine_barrier()
    assert self.sems is not None
    popped = nc._tile_sem_poison_stack.pop()
    assert popped is self._sem_poison
    nc.clear_and_free_semaphores(list(self.sems.allocated().values()))
    nc.all_engine_barrier()


tile.TileContext._drain_and_barrier = _patched_drain_and_barrier

# The same 1-wait-per-instruction walrus limit applies to every engine's
# instructions. Legalize at BIR-serialization time: hoist all but the last
# wait of a multi-wait instruction onto fresh single-wait Drains inserted
# just before it on the same engine.
_orig_to_json_bytes = bass.Bass.to_json_bytes


def _legalized_to_json_bytes(self):
    import orjson

    j = orjson.loads(_orig_to_json_bytes(self))
    ctr = 0
    for fn in j["functions"]:
        for bb in fn["blocks"]:
            new_insts = []
            for ins in bb["instructions"]:
                si = ins.get("sync_info")
                waits = si.get("on_wait") if si else None
                if waits and len(waits) > 1:
                    for w in waits[:-1]:
                        ctr += 1
                        new_insts.append(
                            {
                                "debug": ins.get("debug"),
                                "engine": ins["engine"],
                                "ins": [],
                                "outs": [],
                                "name": f"I-wf{ctr}",
                                "opcode": "EventSemaphore",
                                "sync_info": {"on_update": [], "on_wait": [w]},
                            }
                        )
                    si["on_wait"] = [waits[-1]]
                new_insts.append(ins)
            bb["instructions"] = new_insts
    return orjson.dumps(j)


bass.Bass.to_json_bytes = _legalized_to_json_bytes


def build_program():
    nc = bass.Bass("TRN2", target_bir_lowering=False, debug=False)

    xt_d = nc.declare_dram_parameter("xt", [E, T], DT.bfloat16, isOutput=False)
    wqk_d = nc.declare_dram_parameter("wqk", [E, 2 * FL], DT.bfloat16, isOutput=False)
    wv_d = nc.declare_dram_parameter("wv", [E, FL], DT.bfloat16, isOutput=False)
    wproj_d = nc.declare_dram_parameter("wproj", [4, 128, E], DT.bfloat16, isOutput=False)
    mask_d = nc.declare_dram_parameter("mask", [4, 128, 512], DT.bfloat16, isOutput=False)
    y_d = nc.declare_dram_parameter("y", [T, E], DT.float32, isOutput=True)

    NT = T // 128  # 8 tiles of 128

    with tile.TileContext(nc) as tc:
        with (
            tc.tile_pool(name="persist", bufs=1) as persist,
            tc.tile_pool(name="work", bufs=3) as work,
            tc.tile_pool(name="pP", bufs=12) as pP,
            tc.tile_pool(name="psMM", bufs=6, space="PSUM") as psMM,
            tc.tile_pool(name="psOT", bufs=2, space="PSUM") as psOT,
        ):
            # ---- persistent SBUF tensors -------------------------------
            # q: 4 pair tiles [128,T] (head 2i rows 0:64, head 2i+1 rows 64:128)
            # k: 8 zero-padded tiles [128,T] (head h's K at its pair offset,
            #    sibling rows zeroed) so S matmuls run as clean K=128 shapes.
            qq_sb = [persist.tile([128, T], DT.bfloat16, tag=f"qq{i}", name=f"qq{i}") for i in range(4)]
            kp_sb = [persist.tile([128, T], DT.bfloat16, tag=f"kp{h}", name=f"kp{h}") for h in range(8)]
            # v: [128, 8*128]; head block h = 64 V cols + ones col + 63 zero
            #    cols, so O matmuls run as clean M=128 shapes.
            vp_sb = [persist.tile([128, HL * 128], DT.bfloat16, tag=f"vp{i}", name=f"vp{i}") for i in range(NT)]
            ot_sb = [persist.tile([128, T], DT.bfloat16, tag=f"ot{p}", name=f"ot{p}") for p in range(4)]
            wproj_sb = [persist.tile([128, E], DT.bfloat16, tag=f"wp{p}", name=f"wp{p}") for p in range(4)]
            mask_sb = [persist.tile([128, 512], DT.bfloat16, tag=f"mk{k}", name=f"mk{k}") for k in range(4)]
            r_bcast = persist.tile([128, T], DT.float32, tag="r_bcast", name="r_bcast")
            rT_sb = persist.tile([128, 8], DT.float32, tag="rT", name="rT")
            ones_col = persist.tile([128, 1], DT.float32, tag="ones_col", name="ones_col")
            ones_row = persist.tile([1, 128], DT.float32, tag="ones_row", name="ones_row")
            ones_r = persist.tile([1, 128], DT.float32r, tag="ones_r", name="ones_r")
            one_sc = persist.tile([1, 1], DT.float32, tag="one_sc", name="one_sc")
            r_sb = persist.tile([1, T], DT.float32, tag="r_sb", name="r_sb")
            r_r = persist.tile([1, T], DT.float32r, tag="r_r", name="r_r")
            s_sb = persist.tile([1, T], DT.float32, tag="s_sb", name="s_sb")
            eps_t = persist.tile([1, 1], DT.float32, tag="eps_t", name="eps_t")
            nc.vector.memset(eps_t, float(EPS))
            nc.vector.memset(ones_col, 1.0)
            nc.vector.memset(ones_row, 1.0)
            nc.vector.tensor_copy(ones_r, ones_row)
            nc.vector.memset(one_sc, 1.0)


            xt_sb = [persist.tile([128, T], DT.bfloat16, tag=f"xt{i}", name=f"xt{i}") for i in range(8)]
            wqk_sb = [persist.tile([128, 2 * FL], DT.bfloat16, tag=f"wqk{i}", name=f"wqk{i}") for i in range(8)]
            wv_sb = [persist.tile([128, FL], DT.bfloat16, tag=f"wv{i}", name=f"wv{i}") for i in range(8)]
            sq = [persist.tile([128, T], DT.float32, tag=f"sq{i}", name=f"sq{i}") for i in range(8)]

            # ---- DMA in: x first (gates the r path), then weights.
            # Issues spread across engines so triggers don't serialize;
            # Scalar issues none so its ACT table load runs immediately.
            # x/wqk tiles split in half so two queues carry each (latency).
            for i in range(8):
                for half in range(2):
                    rows = slice(i * 128 + 64 * half, i * 128 + 64 * half + 64)
                    srows = slice(64 * half, 64 * half + 64)
                    nc.sync.dma_start(out=xt_sb[i][srows, :], in_=xt_d[rows, :])
            for i in range(8):
                for half in range(2):
                    rows = slice(i * 128 + 64 * half, i * 128 + 64 * half + 64)
                    srows = slice(64 * half, 64 * half + 64)
                    nc.gpsimd.dma_start(out=wqk_sb[i][srows, :], in_=wqk_d[rows, :])
            # K pads zeroed on Pool before the first K evictions need them
            for h in range(8):
                pad = slice(0, 64) if h % 2 else slice(64, 128)
                nc.gpsimd.memset(kp_sb[h][pad, :], 0.0)
            for i in range(8):
                nc.gpsimd.dma_start(out=wv_sb[i], in_=wv_d[i * 128 : (i + 1) * 128, :])
            # V' block: ones col at 64, zero cols 65:128
            for i in range(NT):
                v3 = vp_sb[i].rearrange("p (h c) -> p h c", h=HL)
                nc.gpsimd.memset(v3[:, :, 64:65], 1.0)
                nc.gpsimd.memset(v3[:, :, 65:128], 0.0)
            for p in range(4):
                nc.sync.dma_start(out=wproj_sb[p], in_=wproj_d[p])
            for k in range(4):
                nc.sync.dma_start(out=mask_sb[k], in_=mask_d[k])

            # ---- norm stats: sumsq -> r (row) -> r_bcast + rT ----------
            # squares split Scalar/Vector so the r path finishes sooner
            for i in range(8):
                if i % 2 == 0:
                    nc.scalar.square(sq[i], xt_sb[i])
                else:
                    nc.vector.tensor_tensor(
                        sq[i], xt_sb[i], xt_sb[i], mybir.AluOpType.mult
                    )
            for step in (1, 2, 4):
                for i in range(0, 8, 2 * step):
                    nc.vector.tensor_add(sq[i], sq[i], sq[i + step])
            acc = sq[0]

            # first 6 QKV accumulation groups emitted ei-major: each matmul
            # only needs its own (wqk, x) e-tile, so the PE starts as DMAs
            # land. Evictions wait on r_bcast and come later.
            wave1 = [(0, 0), (0, 1), (4, 0), (4, 1), (1, 0), (5, 0)]
            wave_ps = {
                fn: psMM.tile([128, 512], DT.float32, tag="mm", name="qkps")
                for fn in wave1
            }
            for ei in range(8):
                for fi, n in wave1:
                    nc.tensor.matmul(
                        wave_ps[(fi, n)],
                        wqk_sb[ei][:, fi * 128 : (fi + 1) * 128],
                        xt_sb[ei][:, n * 512 : (n + 1) * 512],
                        start=(ei == 0),
                        stop=(ei == 7),
                    )

            for n in range(2):
                ssq = psOT.tile([128, 512], DT.float32, tag="ot", name=f"ssq{n}")
                nc.tensor.matmul(
                    ssq[0:1, :], ones_col, acc[:, n * 512 : (n + 1) * 512],
                    start=True, stop=True,
                )
                half = slice(n * 512, (n + 1) * 512)
                # r = 1/sqrt(mean+eps) = exp(-0.5*ln(mean+eps)); ln+exp share
                # one ACT table set (natural_log_exp_and_others), no reloads
                nc.scalar.activation(
                    s_sb[0:1, half], ssq[0:1, :], AF.Ln, bias=eps_t, scale=1.0 / E
                )
                nc.scalar.activation(
                    r_sb[0:1, half], s_sb[0:1, half], AF.Exp, scale=-0.5
                )
                nc.vector.tensor_copy(r_r[0:1, half], r_sb[0:1, half])
                rbp = psOT.tile([128, 512], DT.float32, tag="ot", name=f"rbp{n}")
                nc.tensor.matmul(rbp, ones_r, r_r[0:1, half], start=True, stop=True)
                nc.vector.tensor_copy(r_bcast[:, half], rbp)

            # ---- QKV projection: qk^T tiles [f=128, t], r folded at evict
            def evict_qk(fi, n, ps):
                half = slice(n * 512, (n + 1) * 512)
                if fi < 4:
                    nc.vector.tensor_mul(qq_sb[fi][:, half], ps, r_bcast[:, half])
                else:
                    # split the K pair into its two zero-padded tiles
                    for par in range(2):
                        rows = slice(64 * par, 64 * par + 64)
                        nc.vector.tensor_mul(
                            kp_sb[2 * (fi - 4) + par][rows, half],
                            ps[rows, :],
                            r_bcast[rows, half],
                        )

            for fi, n in wave1:
                evict_qk(fi, n, wave_ps[(fi, n)])

            # transpose r to per-t-tile columns: rT[p, i] = r[i*128+p]
            rtp = psOT.tile([128, 512], DT.float32, tag="ot", name="rtp")
            for i in range(8):
                nc.tensor.transpose(
                    rtp[:, i : i + 1], r_sb[0:1, i * 128 : (i + 1) * 128], one_sc
                )
            nc.vector.tensor_copy(rT_sb, rtp[:, 0:8])
            for fi, n in [(1, 1), (5, 1), (2, 0), (2, 1), (6, 0), (6, 1),
                          (3, 0), (3, 1), (7, 0), (7, 1)]:
                if True:
                    ps = psMM.tile([128, 512], DT.float32, tag="mm", name="qkps")
                    for ei in range(8):
                        nc.tensor.matmul(
                            ps,
                            wqk_sb[ei][:, fi * 128 : (fi + 1) * 128],
                            xt_sb[ei][:, n * 512 : (n + 1) * 512],
                            start=(ei == 0),
                            stop=(ei == 7),
                        )
                    evict_qk(fi, n, ps)

            # ---- V natural [t=128, 512], rT folded at evict ------------
            for ti in range(NT):
                ps = psMM.tile([128, 512], DT.float32, tag="mm", name="vps")
                for ei in range(8):
                    nc.tensor.matmul(
                        ps,
                        xt_sb[ei][:, ti * 128 : (ti + 1) * 128],
                        wv_sb[ei],
                        start=(ei == 0),
                        stop=(ei == 7),
                    )
                nc.vector.tensor_scalar_mul(
                    vp_sb[ti].rearrange("p (h c) -> p h c", h=HL)[:, :, 0:64],
                    ps.rearrange("p (h c) -> p h c", h=HL),
                    rT_sb[:, ti : ti + 1],
                )

            # ---- attention + out-proj, software-pipelined --------------
            def s_block(b, h):
                """S^T tiles + EXP for head h, query half b. Returns p-tiles."""
                nj = 4 * b + 4
                qt = qq_sb[h // 2]  # full pair tile; kp zeros kill the sibling
                kt = kp_sb[h]
                out = []
                for j in range(nj):
                    k = j - 4 * b  # >= 0 -> diagonal-band tile
                    st = psMM.tile([128, 512], DT.float32, tag="mm", name="st")
                    if k >= 0:
                        W = 512 - 128 * k
                        # prefill additive causal mask, accumulate S on top
                        nc.vector.tensor_copy(st[:, 0:W], mask_sb[k][:, 128 * k : 512])
                        nc.tensor.matmul(
                            st[:, 0:W],
                            kt[:, j * 128 : (j + 1) * 128],
                            qt[:, b * 512 + 128 * k : (b + 1) * 512],
                            start=False,
                            stop=True,
                            skip_group_check=True,
                        )
                    else:
                        W = 512
                        nc.tensor.matmul(
                            st,
                            kt[:, j * 128 : (j + 1) * 128],
                            qt[:, b * 512 : (b + 1) * 512],
                            start=True,
                            stop=True,
                        )
                    p_t = pP.tile([128, 512], DT.bfloat16, tag="p_t", name="p_t")
                    nc.scalar.activation(p_t[:, 0:W], st[:, 0:W], AF.Exp, scale=0.125)
                    out.append((p_t, W))
                return out

            def o_block(b, h, ptiles):
                """O^T accumulation (rows 0:64 dims, row 64 denominator)."""
                nj = len(ptiles)
                ot = psOT.tile([128, 512], DT.float32, tag="ot", name="ot")
                for j, (p_t, W) in enumerate(ptiles):
                    nc.tensor.matmul(
                        ot[:, 512 - W : 512],
                        vp_sb[j][:, h * 128 : (h + 1) * 128],
                        p_t[:, 0:W],
                        start=(j == 0),
                        stop=(j == nj - 1),
                        skip_group_check=True,
                    )
                return ot

            def norm_block(b, h, ot):
                # inv = exp(-ln(denom)): ln row on ACT, K=1 broadcast on PE,
                # exponentiated (negated via scale) during the PSUM->SBUF
                # eviction on ACT, then one DVE mul.
                base = 64 * (h % 2)
                trow = work.tile([1, 512], DT.float32r, tag="invr", name="invr")
                nc.scalar.activation(trow, ot[64:65, :], AF.Ln)
                ibp = psMM.tile([128, 512], DT.float32, tag="mm", name="ibp")
                nc.tensor.matmul(
                    ibp[0:64, :], ones_r[0:1, 0:64], trow, start=True, stop=True
                )
                invb = work.tile([64, 512], DT.float32, tag="invb", name="invb")
                nc.scalar.activation(invb, ibp[0:64, :], AF.Exp, scale=-1.0)
                nc.vector.tensor_mul(
                    ot_sb[h // 2][base : base + 64, b * 512 : (b + 1) * 512],
                    ot[0:64, :],
                    invb,
                )

            def c_block(ti, chunks=1):
                # chunks>1 splits the column range so the tail copy+DMA of
                # the final tiles overlaps their remaining matmuls
                cw = 512 // chunks
                for n in range(2):
                    for c in range(chunks):
                        cs = slice(c * cw, (c + 1) * cw)
                        ps = psMM.tile([128, 512], DT.float32, tag="mm", name="yps")
                        for p in range(4):
                            nc.tensor.matmul(
                                ps[:, 0:cw],
                                ot_sb[p][:, ti * 128 : (ti + 1) * 128],
                                wproj_sb[p][:, n * 512 + c * cw : n * 512 + (c + 1) * cw],
                                start=(p == 0),
                                stop=(p == 3),
                            )
                        ysb = work.tile([128, 512], DT.float32, tag="ysb", name="ysb")
                        nc.vector.tensor_copy(ysb[:, 0:cw], ps[:, 0:cw])
                        nc.gpsimd.dma_start(
                            out=y_d[ti * 128 : (ti + 1) * 128, n * 512 : (n + 1) * 512][:, cs],
                            in_=ysb[:, 0:cw],
                        )

            seq = [(0, h) for h in range(HL)] + [(1, h) for h in range(HL)]
            pend_o = []
            pend_n = []
            for idx in range(len(seq) + 2):
                # norm first so its Ln isn't queued behind this idx's EXPs
                if idx >= 2 and pend_n:
                    norm_block(*pend_n.pop(0))
                if idx == 10:  # all b=0 heads normalized by idx 9
                    for ti in range(4):
                        c_block(ti)
                if idx < len(seq):
                    b, h = seq[idx]
                    pend_o.append((b, h, s_block(b, h)))
                if idx >= 1 and pend_o:
                    b, h, ptiles = pend_o.pop(0)
                    pend_n.append((b, h, o_block(b, h, ptiles)))
            for ti in range(4, 8):
                c_block(ti, chunks=2 if ti == 7 else 1)
    return nc


def _make_masks():
    p = np.arange(128)[:, None]
    c = np.arange(512)[None, :]
    return np.stack(
        [np.where(c >= p + 128 * k, 0.0, NEG) for k in range(4)]
    ).astype(BF16)


def prep_inputs(x, scale, w_qkv, w_proj):
    """Per-core input dict list. Core c: batch c//2, head-group c%2."""
    x = np.asarray(x, np.float32)
    scale = np.asarray(scale, np.float32)
    w_qkv = np.asarray(w_qkv, np.float32)
    w_proj = np.asarray(w_proj, np.float32)
    ws = w_qkv * scale[None, :]  # fold RMSNorm scale into the weights
    masks = _make_masks()
    in_maps = []
    for c in range(N_CORES):
        b, g = c // 2, c % 2
        rows = slice(g * FL, (g + 1) * FL)
        wq = ws[0:E][rows]
        wk = ws[E : 2 * E][rows]
        wv = ws[2 * E : 3 * E][rows]
        wproj_t = np.ascontiguousarray(w_proj[:, rows].T).astype(BF16)  # [FL, E]
        in_maps.append(
            {
                "xt": np.ascontiguousarray(x[b].T).astype(BF16),
                "wqk": np.ascontiguousarray(np.concatenate([wq, wk], 0).T).astype(BF16),
                "wv": np.ascontiguousarray(wv.T).astype(BF16),
                "wproj": wproj_t.reshape(4, 128, E),
                "mask": masks,
            }
        )
    return in_maps


_CACHED_NC = None


def kernel(x, scale, w_qkv, w_proj):
    global _CACHED_NC
    if _CACHED_NC is None:
        _CACHED_NC = build_program()
    in_maps = prep_inputs(x, scale, w_qkv, w_proj)
    res = run_bass_kernel_spmd(_CACHED_NC, in_maps, list(range(N_CORES)))
    out = np.zeros((B, T, E), np.float32)
    for c in range(N_CORES):
        out[c // 2] += res.results[c]["y"]
    return out


if __name__ == "__main__":
    rng = np.random.default_rng(0)
    x = rng.standard_normal((B, T, E), dtype=np.float32)
    scale = np.ones(E, np.float32)
    w_qkv = rng.standard_normal((3 * E, E), dtype=np.float32) / 32
    w_proj = rng.standard_normal((E, E), dtype=np.float32) / 32
    y = kernel(x, scale, w_qkv, w_proj)
    print("ran", y.shape, y.dtype, np.abs(y).mean())
